# revision 58
# baseline (speedup 1.0000x reference)
"""Trainium2 Bass kernel for nn_MultiHeadAttention (B=2, S=4096, D=512, H=8).

Computes: q/k/v = relu(x@W+b) per head, softmax(q k^T / sqrt(64)) v,
out = relu(concat_heads @ Wo + bo).

Sharding: 8 cores = 2 (batch) x 4 (query-slice).  Each core computes full
K/V projections for its batch (redundant across the 4 q-slice cores) and
attention + output projection for its 1024-row query slice.  No collectives;
the host concatenates the 8 output slices.

Host-side prep (part of the sharding/layout step, not device compute):
x and Wq/Wk/Wv are cast to fp8 e4m3 (x also transposed feature-major),
Wo to bf16, all pre-tiled to exact SBUF layout.

Per-core kernel:
  - Q/K/V projections are fp8 DoubleRow matmuls (2 fp8 weights/cell,
    contraction 256 = 128 partitions x 2 kt): half the matmul+LDW count
    of bf16.  Bias+relu fused on DVE -> bf16 K^T/Q^T, fp8 V.
  - scores^T = K^T_h.T @ Q^T_h per (head, ktile): bf16, K=64 contraction;
    the two heads of a pair run concurrently in different PE row-groups
    (emission interleaves A/B per ktile).
  - exp: the two ktiles of a pair go to DIFFERENT engines so they run
    concurrently: even ktile on ACT (table exp, scale=1/8, bias=-2ln2,
    fp8 out), odd ktile on DVE via a Schraudolph bit-trick exp -- one
    tensor_scalar: e4m3(exp(s/8)/4) ~= bitcast_e4m3(int8(A8*s + B8))
    (DVE rounds fp32->int8 to nearest; ~7.5%/elem max err that cancels
    through the softmax ratio to ~1e-3 end-to-end; the /4 keeps
    exp(6.6) in e4m3 range and cancels in the ratio too).  Every 3rd
    odd ktile goes to ACT instead to balance engine load.  Both write
    halves of a shared per-ktpair fp8 tile pT2 [p, ko=2, (A|B)].
  - U^T[65, q] = DoubleRow fp8 matmul per (head, ktpair): lhsT =
    V8[:, 2ktp:2ktp+2, h, 0:65] (ones column 64 = softmax denominator
    row; V8 padded to 66 so the ko step is 16-aligned per the LDW ISA
    rule), rhs = pT2[:, :, h-half]; accumulated over 16 ktpairs in PSUM.
    U matmuls lag one ktpair behind their exps so the in-order PE never
    stalls at a U whose exps are still running.
  - PSUM: scores/projection tiles share one 3-slot rotation ("ps3",
    3 x 2 banks) + 2 psU accumulator banks = all 8 banks; 3 slots keep
    the scores->exp pipeline deep enough to hide exp latency.
  - block end: U copied to SBUF (frees psU), then normalize runs with a
    1-block lag: denominator row copied to partition 0 (custom-DVE ops
    need base partition 0), reciprocal_approx_fast (single DVE op,
    ~5x faster than the iterative reciprocal), gpsimd partition
    broadcast, DVE multiply into feature-major O^T.
  - out = relu(O^T.T @ Wo + bo) in bf16 (fp8 here would cost ~1.8e-2
    rel err -- no softmax cancellation after the output projection);
    bias via ones-row matmul; relu on ACT; DMA to HBM.  OT1-gating and
    partial early chains keep the PE busy through the tail.
"""

import numpy as np
import ml_dtypes

import concourse.bass as bass
import concourse.mybir as mybir
import concourse.tile as tile
from concourse import bacc
from concourse import bass_utils

F32 = mybir.dt.float32
BF16 = mybir.dt.bfloat16
I16 = mybir.dt.int16
I8 = mybir.dt.int8
F8 = mybir.dt.float8e4
AF = mybir.ActivationFunctionType
ALU = mybir.AluOpType
PM_DR = mybir.MatmulPerfMode.DoubleRow

LN2 = float(np.log(2.0))
# Schraudolph-style exp on DVE, direct to fp8 e4m3 (IEEE, bias 7, max 240):
# e4m3(exp(s/8)/4) ~= bitcast_e4m3(int8(A8*s + B8)).  The DVE tensor_scalar
# rounds fp32->int8 to nearest (HW-verified); max rel err ~7.5%/elem, which
# cancels to ~1e-3 end-to-end through the softmax ratio (numerator and
# denominator share the fp8 pT).  The /4 scale keeps exp(6.6) inside e4m3
# range and cancels in the softmax ratio too.
EXP_A8 = 1.0 / LN2            # 8 * (1/8) / ln2
EXP_B8 = 56.0 - 16.0 - 0.344  # 7*8 bias, -16 for the /4 scale, centering
P = 128
D = 512
H = 8
DH = 64
DT = D // P  # 4 (also = number of head pairs)
B = 2
S = 4096
NCORES = 8
QSPLIT = 4
SQ_FULL = S // QSPLIT  # 1024 query rows per core
QC = 512               # q-chunk (matmul free dim / PSUM bank width)


def build_mha(sk=S, sq=SQ_FULL, skip_vbias=False):
    """Build the SPMD Bass program (identical on all cores).

    All inputs arrive pre-tiled by the host into exact SBUF layout
    ([128 partitions, contiguous free bytes]) so every load is a max-packet
    linear DMA."""
    nc = bacc.Bacc("TRN2", target_bir_lowering=False, debug=False,
                   num_devices=NCORES)

    xT_d = nc.dram_tensor("xT_bf", (P, DT * sk), F8,
                          kind="ExternalInput").ap()  # chunk-major, see prep
    xqT_d = nc.dram_tensor("xqT_bf", (P, DT * sq), F8,
                           kind="ExternalInput").ap()
    w_dram = {}
    for n in ("wq", "wk", "wv"):
        w_dram[n] = nc.dram_tensor(n, (P, DT * D), F8,
                                   kind="ExternalInput").ap()
    w_dram["wo"] = nc.dram_tensor("wo", (P, DT * D), BF16,
                                  kind="ExternalInput").ap()
    b_dram = {
        "bq": nc.dram_tensor("bq", (P, DT), F32, kind="ExternalInput").ap(),
        "bk": nc.dram_tensor("bk", (P, DT), F32, kind="ExternalInput").ap(),
        "bv": nc.dram_tensor("bv", (1, D), BF16, kind="ExternalInput").ap(),
        "bo": nc.dram_tensor("bo", (1, D), BF16, kind="ExternalInput").ap(),
    }
    out = nc.dram_tensor("out", (sq, D), F32, kind="ExternalOutput").ap()

    with tile.TileContext(nc) as tc:
        _build_tile(tc, xT_d, xqT_d, w_dram, b_dram, out, sk, sq,
                    skip_vbias)

    nc.compile()
    return nc


def _build_tile(tc, xT_d, xqT_d, w_dram, b_dram, out, sk, sq,
                skip_vbias=False):
    nc = tc.nc
    SK_T = sk // P            # ktiles of the key/value sequence
    SQ_T = sq // P
    NQC = sq // QC            # q chunks per core
    CH = min(4, SK_T)         # stiles per projection chunk
    NCH = SK_T // CH
    KG = 1                    # ktiles per exp group

    with (
        tc.tile_pool(name="singles", bufs=1) as singles,
        tc.tile_pool(name="work", bufs=3) as work,
        tc.tile_pool(name="psum", bufs=2, space="PSUM") as psum,
    ):
        # ---- startup: only what Q-proj pair 0 needs, first ----
        w_bf = {}
        w_bf["wq"] = singles.tile([P, DT, D], F8, name="wq_bf")
        nc.sync.dma_start(w_bf["wq"], w_dram["wq"].rearrange(
            "p (t n) -> p t n", t=DT))
        b_col = {}
        b_col["bq"] = singles.tile([P, DT], F32, name="bq_col")
        nc.sync.dma_start(b_col["bq"], b_dram["bq"])
        xTq = singles.tile([P, DT, sq], F8)
        nc.sync.dma_start(xTq, xqT_d.rearrange("p (t s) -> p t s", t=DT))

        QT = singles.tile([P, DT, sq], BF16)

        def qproj(j, nq):
            psQ = psum.tile([P, QC], F32, tag="ps3", bufs=3, name="psQ")
            for t in range(DT // 2):
                nc.tensor.matmul(
                    psQ, w_bf["wq"][:, 2 * t:2 * t + 2, j * P:(j + 1) * P],
                    xTq[:, 2 * t:2 * t + 2, nq * QC:(nq + 1) * QC],
                    start=(t == 0), stop=(t == DT // 2 - 1),
                    perf_mode=PM_DR)
            nc.vector.tensor_scalar(
                QT[:, j, nq * QC:(nq + 1) * QC], psQ,
                b_col["bq"][:, j:j + 1], 0.0, op0=ALU.add, op1=ALU.max)

        qproj(0, 0)
        if NQC > 1:
            qproj(0, 1)

        # ---- K-proj deps next (attention can start before V exists) ----
        b_row = {}
        w_bf["wk"] = singles.tile([P, DT, D], F8, name="wk_bf")
        nc.sync.dma_start(w_bf["wk"], w_dram["wk"].rearrange(
            "p (t n) -> p t n", t=DT))
        b_col["bk"] = singles.tile([P, DT], F32, name="bk_col")
        nc.sync.dma_start(b_col["bk"], b_dram["bk"])
        CHP = CH * P
        xT = singles.tile([P, NCH, DT, CHP], F8)
        xT_src = xT_d.rearrange("p (n t s) -> p n t s", n=NCH, t=DT)
        nc.sync.dma_start(xT[:, 0], xT_src[:, 0])
        for n in ("wv", "wo"):
            wb = singles.tile([P, DT, D], F8 if n == "wv" else BF16,
                              name=f"{n}_bf")
            nc.sync.dma_start(wb, w_dram[n].rearrange(
                "p (t n) -> p t n", t=DT))
            w_bf[n] = wb
            if n == "wv":
                br = singles.tile([1, D], BF16, name="bv_row")
                nc.sync.dma_start(br, b_dram["bv"])
                b_row["bv"] = br
        br = singles.tile([1, D], BF16, name="bo_row")
        nc.sync.dma_start(br, b_dram["bo"])
        b_row["bo"] = br

        # ---- persistent SBUF tensors ----
        xT1 = singles.tile([1, sk], BF16)
        nc.vector.memset(xT1, 1.0)
        KT = singles.tile([P, DT, sk], BF16)
        # V in fp8 e4m3 for the DoubleRow U matmul.  Layout [p, st, h, 66]:
        # col 64 = ones (denominator row), col 65 = pad so the DoubleRow
        # ktile-pair step (8*66 = 528 elems) is 16-aligned per the LDW ISA
        # restriction.
        MV = DH + 2
        V8 = singles.tile([P, SK_T, H, MV], F8, name="V8")
        nc.vector.memset(V8[:, :, :, DH:DH + 1], 1.0)
        nl2m = singles.tile([P, 1], F32, name="nl2m")
        nc.vector.memset(nl2m, -2.0 * LN2)  # ACT exp bias: the /4 scale
        OT = singles.tile([P, DT, sq], BF16)
        OT1 = singles.tile([1, sq], BF16)
        nc.vector.memset(OT1, 1.0)

        # PSUM tags: "proj" 2x1 banks, "scores" 2x2 banks, "psU" 2x1 = 8
        def vproj(st):
            n, si = st // CH, st % CH
            psV = psum.tile([P, D], F32, tag="ps3", bufs=3, name="psV")
            for t in range(DT // 2):
                nc.tensor.matmul(
                    psV, xT[:, n, 2 * t:2 * t + 2, si * P:(si + 1) * P],
                    w_bf["wv"][:, 2 * t:2 * t + 2, :],
                    start=(t == 0),
                    stop=(skip_vbias and t == DT // 2 - 1),
                    perf_mode=PM_DR)
            if not skip_vbias:
                nc.tensor.matmul(psV, xT1[:, st * P:(st + 1) * P],
                                 b_row["bv"], start=False, stop=True)
            nc.vector.tensor_scalar_max(
                V8[:, st, :, 0:DH],
                psV.rearrange("p (h d) -> p h d", h=H), 0.0)

        def kproj(j, n):
            psK = psum.tile([P, CH * P], F32, tag="ps3", bufs=3, name="psK")
            for t in range(DT // 2):
                nc.tensor.matmul(
                    psK, w_bf["wk"][:, 2 * t:2 * t + 2, j * P:(j + 1) * P],
                    xT[:, n, 2 * t:2 * t + 2, :],
                    start=(t == 0), stop=(t == DT // 2 - 1),
                    perf_mode=PM_DR)
            nc.vector.tensor_scalar(
                KT[:, j, n * CH * P:(n + 1) * CH * P], psK,
                b_col["bk"][:, j:j + 1], 0.0, op0=ALU.add, op1=ALU.max)

        exp_state = {"g": 0}

        def attn_qk_exp(j, qc, ktp, pt_tag="pT", pt_bufs=9, eng=None):
            """Scores + exp for ktile pair (2*ktp, 2*ktp+1) x 2 heads.
            Emission matches the pre-DoubleRow kernel: per ktile, one psS
            [P, A|B] with the two heads' matmuls adjacent (PE row-group
            pairing), one exp op [P, 1024].  The exp output lands in a
            shared per-ktpair fp8 tile pT2 [P, ko=2, (A|B)] so each head's
            DoubleRow rhs is the strided view pT2[:, :, h-half]."""
            q0 = qc * QC
            # byte-interleaved pair layout [p, q, ko]: the DR rhs stream
            # fetches the 2 fp8 values of a cell as one 16-bit read
            pT2 = work.tile([P, 2 * QC, 2], F8, tag=pt_tag,
                            bufs=pt_bufs, name="pT2")
            for i in (0, 1):
                kt = 2 * ktp + i
                psS = psum.tile([P, 2 * QC], F32, tag="ps3", bufs=3,
                                name="psS")
                nc.tensor.matmul(
                    psS[:, 0:QC],
                    KT[0:DH, j, kt * P:(kt + 1) * P],
                    QT[0:DH, j, q0:q0 + QC], start=True, stop=True)
                nc.tensor.matmul(
                    psS[:, QC:2 * QC],
                    KT[DH:P, j, kt * P:(kt + 1) * P],
                    QT[DH:P, j, q0:q0 + QC], start=True, stop=True)
                if eng is not None:
                    e = eng
                elif i == 0:
                    e = "act"
                else:
                    g = exp_state["g"]
                    exp_state["g"] = g + 1
                    e = "act" if (g % 3) == 2 else "dve"
                if e == "dve":
                    nc.vector.tensor_scalar(
                        pT2[:, :, i].bitcast(I8), psS,
                        EXP_A8, EXP_B8, op0=ALU.mult, op1=ALU.add)
                else:
                    nc.scalar.activation(pT2[:, :, i], psS, AF.Exp,
                                         scale=0.125, bias=nl2m)
            return pT2

        def attn_u(j, ktp, pT2, psU_A, psU_B):
            """DoubleRow fp8 matmul: contraction over both ktiles of the
            pair (128 partitions x ko=2)."""
            first, last = (ktp == 0), (ktp == SK_T // 2 - 1)
            for h2, psU in ((0, psU_A), (1, psU_B)):
                nc.tensor.matmul(
                    psU, V8[:, 2 * ktp:2 * ktp + 2, 2 * j + h2, 0:DH + 1],
                    pT2[:, h2 * QC:(h2 + 1) * QC, :].transpose([0, 2, 1]),
                    start=first, stop=last, perf_mode=PM_DR)

        def attn_group(j, qc, ktp, psU_A, psU_B):
            pT2 = attn_qk_exp(j, qc, ktp)
            attn_u(j, ktp, pT2, psU_A, psU_B)

        def attn_finish_copies(psU_A, psU_B):
            """Copy U out of PSUM fast — frees both accumulators for the
            next block.  Returns the SBUF copies."""
            ucs = []
            for psU in (psU_A, psU_B):
                uc = work.tile([DH + 1, QC], F32, tag="ucopy", bufs=6,
                               name="uc")
                nc.vector.tensor_copy(uc, psU)
                ucs.append(uc)
            return ucs

        brc_sink = {}

        def normalize_thunks(j, qc, ucs):
            """Per-head softmax normalize emitted later (as fillers inside
            the next block) so its latency hides under ACT-bound stretches."""
            q0 = qc * QC

            def one(uc, h0):
                def t():
                    # custom-DVE ops require base partition 0: copy the
                    # denominator row down before the fast reciprocal
                    d0 = work.tile([1, QC], F32, tag="d0", bufs=2, name="d0")
                    nc.vector.tensor_copy(d0, uc[DH:DH + 1, :])
                    recip = work.tile([1, QC], F32, tag="recip", bufs=4,
                                      name="recip")
                    nc.vector.reciprocal_approx_fast(recip, d0)
                    brc = work.tile([DH, QC], F32, tag="brc", bufs=4,
                                    name="brc")
                    nc.gpsimd.partition_broadcast(brc, recip)
                    nc.vector.tensor_mul(
                        OT[h0:h0 + DH, j, q0:q0 + QC], uc[0:DH, :], brc)
                    brc_sink[(j, qc)] = brc
                return t
            return [one(ucs[0], 0), one(ucs[1], DH)]

        def attn_span(j, qc, ktps, psU, fillers=(), precomputed=()):
            """Emit the ktile-pair groups of one attention block, sprinkling
            `fillers` (deferred work thunks) between groups so the in-order
            PE/DVE do them inside exp-bound stretches.  Returns this block's
            normalize thunks (to be run as fillers of the NEXT block)."""
            fillers = list(fillers)
            for ktp, pT2 in precomputed:
                attn_u(j, ktp, pT2, psU[0], psU[1])
            spacing = max(1, len(ktps) // (len(fillers) + 1))
            gi = 0
            pend = []     # U-DR lags four ktpairs: the in-order PE never
            for ktp in ktps:  # stalls at a U whose exps are still running
                pT2 = attn_qk_exp(j, qc, ktp)
                pend.append((ktp, pT2))
                if len(pend) > 4:
                    kp, pt = pend.pop(0)
                    attn_u(j, kp, pt, psU[0], psU[1])
                gi += 1
                if fillers and gi % spacing == 0:
                    fillers.pop(0)()
            for kp, pt in pend:
                attn_u(j, kp, pt, psU[0], psU[1])
            for f in fillers:
                f()
            if ktps[-1] == SK_T // 2 - 1:
                ucs = attn_finish_copies(psU[0], psU[1])
                return normalize_thunks(j, qc, ucs), ucs
            return [], None

        def new_psU():
            a = psum.tile([DH + 1, QC], F32, tag="psU", name="psU_A")
            b = psum.tile([DH + 1, QC], F32, tag="psU", name="psU_B")
            return (a, b)

        def outproj(qt):
            # bias matmul first: it reads OT1, whose re-write after the last
            # normalize acts as a scheduling gate for the whole chain (the
            # scheduler otherwise hoists these into mid-attention PE-idle
            # slots and stalls on under-modeled reciprocal latency)
            psO = psum.tile([P, D], F32, tag="ps3", bufs=3, name="psO")
            nc.tensor.matmul(psO, OT1[:, qt * P:(qt + 1) * P],
                             b_row["bo"], start=True, stop=False)
            for j in range(DT):
                nc.tensor.matmul(psO, OT[:, j, qt * P:(qt + 1) * P],
                                 w_bf["wo"][:, j, :],
                                 start=False, stop=(j == DT - 1))
            o_sb = work.tile([P, D], F32, tag="osb", bufs=4, name="o_sb")
            nc.scalar.activation(o_sb, psO, AF.Relu)
            nc.sync.dma_start(out[qt * P:(qt + 1) * P, :], o_sb)

        def gate_outproj(blk):
            """No-op rewrite of OT1 (max(1, recip<1) == 1) that depends on
            block `blk`'s normalize chain — gates the outproj chains (which
            start with an OT1-reading bias matmul) behind it, preventing the
            scheduler from hoisting them into mid-attention stalls."""
            brc = brc_sink[blk]
            nc.vector.tensor_scalar(OT1, OT1, brc[0:1, 0:1], None,
                                    op0=ALU.max)

        # ---- chunk loop: x load + V proj + K proj(pair 0) + attn(0, 0) ----
        psU0 = new_psU()
        N_STORE = 12
        store01 = []
        pendq = []   # queue of deferred normalize-thunk lists (2-block lag)
        for n in range(NCH):
            if n > 0:
                nc.sync.dma_start(xT[:, n], xT_src[:, n])
            kproj(0, n)
            kts = list(range(n * CH, (n + 1) * CH))
            ktps = list(range(n * CH // 2, (n + 1) * CH // 2))
            # QK + exp first: ACT can start before V exists (only U needs V)
            pTs = [(ktp, attn_qk_exp(0, 0, ktp)) for ktp in ktps]
            for st in kts:
                vproj(st)
            for ktp, pT2 in pTs:
                attn_u(0, ktp, pT2, psU0[0], psU0[1])
            if NQC > 1 and n < N_STORE:
                # pre-compute one ktile-pair of block (0,1) per chunk into
                # held pTs: fills the otherwise-idle ACT during the PE-bound
                # chunk phase (the U matmuls run later, so no PSUM cost)
                store01.append((n, attn_qk_exp(0, 1, n, pt_tag="pT01",
                                               pt_bufs=N_STORE, eng="act")))
            if kts[-1] == SK_T - 1:
                ucs0 = attn_finish_copies(psU0[0], psU0[1])
                thunks = normalize_thunks(0, 0, ucs0)
        pendq.append(thunks)

        # ---- remaining attention; fillers inside each ACT-bound block are:
        # the previous block's normalize chain + the next block's
        # projections (+ the qc0 half of the output projection during the
        # last block) ----
        blocks = [(0, qc) for qc in range(1, NQC)]
        blocks += [(j, qc) for j in range(1, DT) for qc in range(NQC)]
        owed = {blk: [] for blk in blocks}
        for (j, qc) in blocks:
            if (j, qc) != (0, 1):
                owed[(j, qc)].append(lambda j=j, qc=qc: qproj(j, qc))
            if qc == 0 and j >= 1:
                for n in range(NCH):
                    owed[(j, qc)].append(lambda j=j, n=n: kproj(j, n))
        for f in owed[blocks[0]]:
            f()
        for bi, (j, qc) in enumerate(blocks):
            # projection fillers first; normalize chains run with a 2-block
            # lag so their slow DVE reciprocals never sit near a block
            # boundary (where they would delay the relus feeding the next
            # pair's attention)
            fillers = []
            if bi + 1 < len(blocks):
                fillers += owed[blocks[bi + 1]]
            last = bi == len(blocks) - 1
            if last:
                # flush remaining normalize chains, then gate + emit the qc0
                # half of the output projection so it runs inside this block
                while pendq:
                    fillers += pendq.pop(0)
                if NQC > 1:
                    fillers += [lambda: gate_outproj((DT - 1, 0))]
                    fillers += [lambda qt=qt: outproj(qt)
                                for qt in range(SQ_T // NQC)]
            elif len(pendq) >= 1:
                fillers += pendq.pop(0)
                if bi == len(blocks) - 2 and pendq:
                    fillers += pendq.pop(0)
            psU = new_psU()
            if (j, qc) == (0, 1) and store01:
                thunks, ucs = attn_span(
                    j, qc, list(range(len(store01), SK_T // 2)), psU,
                    fillers, precomputed=store01)
            else:
                thunks, ucs = attn_span(j, qc, list(range(SK_T // 2)), psU,
                                        fillers)
            pendq.append(thunks)
            last_ucs = ucs

        # ---- tail: last block's normalize + remaining output rows ----
        # Two of the final outproj chains are gated only on the last block's
        # PSUM copies (their bias + pairs-0..2 matmuls need nothing newer),
        # so the PE does useful work during the slow reciprocal chain and
        # stays HAM-warm; their pair-3 matmul still waits on the real OT
        # write.  Gate writes go on DVE BEFORE the normalize thunks so they
        # are not queued behind the reciprocals.
        qt_lo = SQ_T // NQC if NQC > 1 else 0
        early = []
        open_psO = []
        if NQC > 1 and last_ucs is not None:
            early = [qt_lo, qt_lo + 1, qt_lo + 2, qt_lo + 3]
            for qt, uc in zip(early, list(last_ucs) * 2):
                nc.vector.tensor_scalar(
                    OT1[:, qt * P:(qt + 1) * P],
                    OT1[:, qt * P:(qt + 1) * P],
                    uc[DH:DH + 1, 0:1], None, op0=ALU.min)
            # partial chains (bias + pairs 0..2): no pair-3 matmul yet, so
            # the in-order PE runs all 8 matmuls during the reciprocals
            # instead of stalling at the first chain's pair-3 wait
            for qt in early:
                psO = psum.tile([P, D], F32, tag="ps3", bufs=3, name="psO")
                nc.tensor.matmul(psO, OT1[:, qt * P:(qt + 1) * P],
                                 b_row["bo"], start=True, stop=False)
                for j in range(DT - 1):
                    nc.tensor.matmul(psO, OT[:, j, qt * P:(qt + 1) * P],
                                     w_bf["wo"][:, j, :],
                                     start=False, stop=False)
                open_psO.append((qt, psO))
        while pendq:
            for f in pendq.pop(0):
                f()
        for qt, psO in open_psO:
            nc.tensor.matmul(psO, OT[:, DT - 1, qt * P:(qt + 1) * P],
                             w_bf["wo"][:, DT - 1, :],
                             start=False, stop=True)
            o_sb = work.tile([P, D], F32, tag="osb", bufs=4, name="o_sb")
            nc.scalar.activation(o_sb, psO, AF.Relu)
            nc.sync.dma_start(out[qt * P:(qt + 1) * P, :], o_sb)
        gate_outproj(blocks[-1])
        for qt in range(qt_lo, SQ_T):
            if qt not in early:
                outproj(qt)


_NC_CACHE = {}


def _get_nc(sk=S, sq=SQ_FULL, skip_vbias=False):
    key = (sk, sq, skip_vbias)
    if key not in _NC_CACHE:
        _NC_CACHE[key] = build_mha(sk, sq, skip_vbias)
    return _NC_CACHE[key]


def _tile_rows(a):
    """[D, n] -> SBUF layout [P, DT*n]: partition p gets rows p, 128+p, ..."""
    Dd, n = a.shape
    t = Dd // P
    return np.ascontiguousarray(
        a.reshape(t, P, n).transpose(1, 0, 2).reshape(P, t * n))


def _tile_chunks(a, chp):
    """[D, sk] -> chunk-major SBUF layout [P, NCH*DT*chp]: per partition,
    sequence chunks outermost so each chunk is one contiguous linear DMA."""
    Dd, sk = a.shape
    t, nch = Dd // P, sk // chp
    return np.ascontiguousarray(
        a.reshape(t, P, nch, chp).transpose(1, 2, 0, 3).reshape(P, -1))


def prep_inputs(x, Wq, bq, Wk, bk, Wv, bv, Wo, bo):
    """Host-side sharding/layout prep: fp8/bf16 casts, feature-major
    transpose, SBUF pre-tiling.  Returns the 8 per-core input maps."""
    bf = ml_dtypes.bfloat16
    f8 = ml_dtypes.float8_e4m3
    x = np.asarray(x, dtype=np.float32)
    shared = {
        "wq": _tile_rows(np.asarray(Wq, np.float32).astype(f8)),
        "wk": _tile_rows(np.asarray(Wk, np.float32).astype(f8)),
        "wv": _tile_rows(np.asarray(Wv, np.float32).astype(f8)),
        "wo": _tile_rows(np.asarray(Wo, np.float32).astype(bf)),
        "bq": np.ascontiguousarray(
            np.asarray(bq, np.float32).reshape(DT, P).T),
        "bk": np.ascontiguousarray(
            np.asarray(bk, np.float32).reshape(DT, P).T),
        "bv": np.asarray(bv, np.float32).astype(bf).reshape(1, D),
        "bo": np.asarray(bo, np.float32).astype(bf).reshape(1, D),
    }
    xT_b = [x[b].T.astype(f8) for b in range(B)]
    xT_tiled = [_tile_chunks(xb, 4 * P) for xb in xT_b]
    in_maps = []
    for c in range(NCORES):
        b, qo = divmod(c, QSPLIT)
        m = dict(shared)
        m["xT_bf"] = xT_tiled[b]
        m["xqT_bf"] = _tile_rows(
            xT_b[b][:, qo * SQ_FULL:(qo + 1) * SQ_FULL])
        in_maps.append(m)
    return in_maps


def kernel(x, Wq, bq, Wk, bk, Wv, bv, Wo, bo, **run_kwargs):
    """Full-input entry point: shards across 8 NeuronCores, returns full out."""
    in_maps = prep_inputs(x, Wq, bq, Wk, bk, Wv, bv, Wo, bo)
    nc = _get_nc(skip_vbias=bool(np.all(np.asarray(bv) == 0)))
    res = bass_utils.run_bass_kernel_spmd(
        nc, in_maps, core_ids=list(range(NCORES)), **run_kwargs)
    full = np.empty((B, S, D), np.float32)
    for c in range(NCORES):
        b, qo = divmod(c, QSPLIT)
        full[b, qo * SQ_FULL:(qo + 1) * SQ_FULL] = res.results[c]["out"]
    if run_kwargs:
        return full, res
    return full



# revision 59
# speedup vs baseline: 1.0079x; 1.0079x over previous
"""Trainium2 Bass kernel for nn_MultiHeadAttention (B=2, S=4096, D=512, H=8).

Computes: q/k/v = relu(x@W+b) per head, softmax(q k^T / sqrt(64)) v,
out = relu(concat_heads @ Wo + bo).

Sharding: 8 cores = 2 (batch) x 4 (query-slice).  Each core computes full
K/V projections for its batch (redundant across the 4 q-slice cores) and
attention + output projection for its 1024-row query slice.  No collectives;
the host concatenates the 8 output slices.

Host-side prep (part of the sharding/layout step, not device compute):
x and Wq/Wk/Wv are cast to fp8 e4m3 (x also transposed feature-major),
Wo to bf16, all pre-tiled to exact SBUF layout.

Per-core kernel:
  - Q/K/V projections are fp8 DoubleRow matmuls (2 fp8 weights/cell,
    contraction 256 = 128 partitions x 2 kt): half the matmul+LDW count
    of bf16.  Bias+relu fused on DVE -> bf16 K^T/Q^T, fp8 V.
  - scores^T = K^T_h.T @ Q^T_h per (head, ktile): bf16, K=64 contraction;
    the two heads of a pair run concurrently in different PE row-groups
    (emission interleaves A/B per ktile).
  - exp: the two ktiles of a pair go to DIFFERENT engines so they run
    concurrently: even ktile on ACT (table exp, scale=1/8, bias=-2ln2,
    fp8 out), odd ktile on DVE via a Schraudolph bit-trick exp -- one
    tensor_scalar: e4m3(exp(s/8)/4) ~= bitcast_e4m3(int8(A8*s + B8))
    (DVE rounds fp32->int8 to nearest; ~7.5%/elem max err that cancels
    through the softmax ratio to ~1e-3 end-to-end; the /4 keeps
    exp(6.6) in e4m3 range and cancels in the ratio too).  Every 3rd
    odd ktile goes to ACT instead to balance engine load.  Both write
    halves of a shared per-ktpair fp8 tile pT2 [p, ko=2, (A|B)].
  - U^T[65, q] = DoubleRow fp8 matmul per (head, ktpair): lhsT =
    V8[:, 2ktp:2ktp+2, h, 0:65] (ones column 64 = softmax denominator
    row; V8 padded to 66 so the ko step is 16-aligned per the LDW ISA
    rule), rhs = pT2[:, :, h-half]; accumulated over 16 ktpairs in PSUM.
    U matmuls lag one ktpair behind their exps so the in-order PE never
    stalls at a U whose exps are still running.
  - PSUM: scores/projection tiles share one 3-slot rotation ("ps3",
    3 x 2 banks) + 2 psU accumulator banks = all 8 banks; 3 slots keep
    the scores->exp pipeline deep enough to hide exp latency.
  - block end: U copied to SBUF (frees psU), then normalize runs with a
    1-block lag: denominator row copied to partition 0 (custom-DVE ops
    need base partition 0), reciprocal_approx_fast (single DVE op,
    ~5x faster than the iterative reciprocal), gpsimd partition
    broadcast, DVE multiply into feature-major O^T.
  - out = relu(O^T.T @ Wo + bo) in bf16 (fp8 here would cost ~1.8e-2
    rel err -- no softmax cancellation after the output projection);
    bias via ones-row matmul; relu on ACT; DMA to HBM.  OT1-gating and
    partial early chains keep the PE busy through the tail.
"""

import numpy as np
import ml_dtypes

import concourse.bass as bass
import concourse.mybir as mybir
import concourse.tile as tile
from concourse import bacc
from concourse import bass_utils

F32 = mybir.dt.float32
BF16 = mybir.dt.bfloat16
I16 = mybir.dt.int16
I8 = mybir.dt.int8
F8 = mybir.dt.float8e4
AF = mybir.ActivationFunctionType
ALU = mybir.AluOpType
PM_DR = mybir.MatmulPerfMode.DoubleRow

LN2 = float(np.log(2.0))
# Schraudolph-style exp on DVE, direct to fp8 e4m3 (IEEE, bias 7, max 240):
# e4m3(exp(s/8)/4) ~= bitcast_e4m3(int8(A8*s + B8)).  The DVE tensor_scalar
# rounds fp32->int8 to nearest (HW-verified); max rel err ~7.5%/elem, which
# cancels to ~1e-3 end-to-end through the softmax ratio (numerator and
# denominator share the fp8 pT).  The /4 scale keeps exp(6.6) inside e4m3
# range and cancels in the softmax ratio too.
EXP_A8 = 1.0 / LN2            # 8 * (1/8) / ln2
EXP_B8 = 56.0 - 16.0 - 0.344  # 7*8 bias, -16 for the /4 scale, centering
P = 128
D = 512
H = 8
DH = 64
DT = D // P  # 4 (also = number of head pairs)
B = 2
S = 4096
NCORES = 8
QSPLIT = 4
SQ_FULL = S // QSPLIT  # 1024 query rows per core
QC = 512               # q-chunk (matmul free dim / PSUM bank width)


def build_mha(sk=S, sq=SQ_FULL, skip_vbias=False):
    """Build the SPMD Bass program (identical on all cores).

    All inputs arrive pre-tiled by the host into exact SBUF layout
    ([128 partitions, contiguous free bytes]) so every load is a max-packet
    linear DMA."""
    nc = bacc.Bacc("TRN2", target_bir_lowering=False, debug=False,
                   num_devices=NCORES)

    xT_d = nc.dram_tensor("xT_bf", (P, DT * sk), F8,
                          kind="ExternalInput").ap()  # chunk-major, see prep
    xqT_d = nc.dram_tensor("xqT_bf", (P, DT * sq), F8,
                           kind="ExternalInput").ap()
    w_dram = {}
    for n in ("wq", "wk", "wv"):
        w_dram[n] = nc.dram_tensor(n, (P, DT * D), F8,
                                   kind="ExternalInput").ap()
    w_dram["wo"] = nc.dram_tensor("wo", (P, DT * D), BF16,
                                  kind="ExternalInput").ap()
    b_dram = {
        "bq": nc.dram_tensor("bq", (P, DT), F32, kind="ExternalInput").ap(),
        "bk": nc.dram_tensor("bk", (P, DT), F32, kind="ExternalInput").ap(),
        "bv": nc.dram_tensor("bv", (1, D), BF16, kind="ExternalInput").ap(),
        "bo": nc.dram_tensor("bo", (1, D), BF16, kind="ExternalInput").ap(),
    }
    out = nc.dram_tensor("out", (sq, D), F32, kind="ExternalOutput").ap()

    with tile.TileContext(nc) as tc:
        _build_tile(tc, xT_d, xqT_d, w_dram, b_dram, out, sk, sq,
                    skip_vbias)

    nc.compile()
    return nc


def _build_tile(tc, xT_d, xqT_d, w_dram, b_dram, out, sk, sq,
                skip_vbias=False):
    nc = tc.nc
    SK_T = sk // P            # ktiles of the key/value sequence
    SQ_T = sq // P
    NQC = sq // QC            # q chunks per core
    CH = min(4, SK_T)         # stiles per projection chunk
    NCH = SK_T // CH
    KG = 1                    # ktiles per exp group

    with (
        tc.tile_pool(name="singles", bufs=1) as singles,
        tc.tile_pool(name="work", bufs=3) as work,
        tc.tile_pool(name="psum", bufs=2, space="PSUM") as psum,
    ):
        # ---- startup: only what Q-proj pair 0 needs, first ----
        w_bf = {}
        w_bf["wq"] = singles.tile([P, DT, D], F8, name="wq_bf")
        nc.sync.dma_start(w_bf["wq"], w_dram["wq"].rearrange(
            "p (t n) -> p t n", t=DT))
        b_col = {}
        b_col["bq"] = singles.tile([P, DT], F32, name="bq_col")
        nc.sync.dma_start(b_col["bq"], b_dram["bq"])
        xTq = singles.tile([P, DT, sq], F8)
        nc.sync.dma_start(xTq, xqT_d.rearrange("p (t s) -> p t s", t=DT))

        QT = singles.tile([P, DT, sq], BF16)

        def qproj(j, nq):
            psQ = psum.tile([P, QC], F32, tag="ps3", bufs=3, name="psQ")
            for t in range(DT // 2):
                nc.tensor.matmul(
                    psQ, w_bf["wq"][:, 2 * t:2 * t + 2, j * P:(j + 1) * P],
                    xTq[:, 2 * t:2 * t + 2, nq * QC:(nq + 1) * QC],
                    start=(t == 0), stop=(t == DT // 2 - 1),
                    perf_mode=PM_DR)
            nc.vector.tensor_scalar(
                QT[:, j, nq * QC:(nq + 1) * QC], psQ,
                b_col["bq"][:, j:j + 1], 0.0, op0=ALU.add, op1=ALU.max)

        qproj(0, 0)
        if NQC > 1:
            qproj(0, 1)

        # ---- K-proj deps next (attention can start before V exists) ----
        b_row = {}
        w_bf["wk"] = singles.tile([P, DT, D], F8, name="wk_bf")
        nc.sync.dma_start(w_bf["wk"], w_dram["wk"].rearrange(
            "p (t n) -> p t n", t=DT))
        b_col["bk"] = singles.tile([P, DT], F32, name="bk_col")
        nc.sync.dma_start(b_col["bk"], b_dram["bk"])
        CHP = CH * P
        xT = singles.tile([P, NCH, DT, CHP], F8)
        xT_src = xT_d.rearrange("p (n t s) -> p n t s", n=NCH, t=DT)
        nc.sync.dma_start(xT[:, 0], xT_src[:, 0])
        for n in ("wv", "wo"):
            wb = singles.tile([P, DT, D], F8 if n == "wv" else BF16,
                              name=f"{n}_bf")
            nc.sync.dma_start(wb, w_dram[n].rearrange(
                "p (t n) -> p t n", t=DT))
            w_bf[n] = wb
            if n == "wv":
                br = singles.tile([1, D], BF16, name="bv_row")
                nc.sync.dma_start(br, b_dram["bv"])
                b_row["bv"] = br
        br = singles.tile([1, D], BF16, name="bo_row")
        nc.sync.dma_start(br, b_dram["bo"])
        b_row["bo"] = br

        # ---- persistent SBUF tensors ----
        xT1 = singles.tile([1, sk], BF16)
        nc.vector.memset(xT1, 1.0)
        KT = singles.tile([P, DT, sk], BF16)
        # V in fp8 e4m3 for the DoubleRow U matmul.  Layout [p, st, h, 66]:
        # col 64 = ones (denominator row), col 65 = pad so the DoubleRow
        # ktile-pair step (8*66 = 528 elems) is 16-aligned per the LDW ISA
        # restriction.
        MV = DH + 2
        V8 = singles.tile([P, SK_T, H, MV], F8, name="V8")
        nc.vector.memset(V8[:, :, :, DH:DH + 1], 1.0)
        nl2m = singles.tile([P, 1], F32, name="nl2m")
        nc.vector.memset(nl2m, -2.0 * LN2)  # ACT exp bias: the /4 scale
        OT = singles.tile([P, DT, sq], BF16)
        OT1 = singles.tile([1, sq], BF16)
        nc.vector.memset(OT1, 1.0)

        # PSUM tags: "proj" 2x1 banks, "scores" 2x2 banks, "psU" 2x1 = 8
        def vproj(st):
            n, si = st // CH, st % CH
            psV = psum.tile([P, D], F32, tag="ps3", bufs=3, name="psV")
            for t in range(DT // 2):
                nc.tensor.matmul(
                    psV, xT[:, n, 2 * t:2 * t + 2, si * P:(si + 1) * P],
                    w_bf["wv"][:, 2 * t:2 * t + 2, :],
                    start=(t == 0),
                    stop=(skip_vbias and t == DT // 2 - 1),
                    perf_mode=PM_DR)
            if not skip_vbias:
                nc.tensor.matmul(psV, xT1[:, st * P:(st + 1) * P],
                                 b_row["bv"], start=False, stop=True)
            nc.vector.tensor_scalar_max(
                V8[:, st, :, 0:DH],
                psV.rearrange("p (h d) -> p h d", h=H), 0.0)

        def kproj(j, n):
            psK = psum.tile([P, CH * P], F32, tag="ps3", bufs=3, name="psK")
            for t in range(DT // 2):
                nc.tensor.matmul(
                    psK, w_bf["wk"][:, 2 * t:2 * t + 2, j * P:(j + 1) * P],
                    xT[:, n, 2 * t:2 * t + 2, :],
                    start=(t == 0), stop=(t == DT // 2 - 1),
                    perf_mode=PM_DR)
            nc.vector.tensor_scalar(
                KT[:, j, n * CH * P:(n + 1) * CH * P], psK,
                b_col["bk"][:, j:j + 1], 0.0, op0=ALU.add, op1=ALU.max)

        exp_state = {"g": 0}

        def attn_qk_exp(j, qc, ktp, pt_tag="pT", pt_bufs=9, eng=None):
            """Scores + exp for ktile pair (2*ktp, 2*ktp+1) x 2 heads.
            Emission matches the pre-DoubleRow kernel: per ktile, one psS
            [P, A|B] with the two heads' matmuls adjacent (PE row-group
            pairing), one exp op [P, 1024].  The exp output lands in a
            shared per-ktpair fp8 tile pT2 [P, ko=2, (A|B)] so each head's
            DoubleRow rhs is the strided view pT2[:, :, h-half]."""
            q0 = qc * QC
            # byte-interleaved pair layout [p, q, ko]: the DR rhs stream
            # fetches the 2 fp8 values of a cell as one 16-bit read
            pT2 = work.tile([P, 2 * QC, 2], F8, tag=pt_tag,
                            bufs=pt_bufs, name="pT2")
            for i in (0, 1):
                kt = 2 * ktp + i
                psS = psum.tile([P, 2 * QC], F32, tag="ps3", bufs=3,
                                name="psS")
                nc.tensor.matmul(
                    psS[:, 0:QC],
                    KT[0:DH, j, kt * P:(kt + 1) * P],
                    QT[0:DH, j, q0:q0 + QC], start=True, stop=True)
                nc.tensor.matmul(
                    psS[:, QC:2 * QC],
                    KT[DH:P, j, kt * P:(kt + 1) * P],
                    QT[DH:P, j, q0:q0 + QC], start=True, stop=True)
                if eng is not None:
                    e = eng
                elif i == 0:
                    e = "act"
                else:
                    g = exp_state["g"]
                    exp_state["g"] = g + 1
                    e = "act" if (g % 3) == 2 else "dve"
                if e == "dve":
                    nc.vector.tensor_scalar(
                        pT2[:, :, i].bitcast(I8), psS,
                        EXP_A8, EXP_B8, op0=ALU.mult, op1=ALU.add)
                else:
                    nc.scalar.activation(pT2[:, :, i], psS, AF.Exp,
                                         scale=0.125, bias=nl2m)
            return pT2

        def attn_u(j, ktp, pT2, psU_A, psU_B):
            """DoubleRow fp8 matmul: contraction over both ktiles of the
            pair (128 partitions x ko=2)."""
            first, last = (ktp == 0), (ktp == SK_T // 2 - 1)
            for h2, psU in ((0, psU_A), (1, psU_B)):
                nc.tensor.matmul(
                    psU, V8[:, 2 * ktp:2 * ktp + 2, 2 * j + h2, 0:DH + 1],
                    pT2[:, h2 * QC:(h2 + 1) * QC, :].transpose([0, 2, 1]),
                    start=first, stop=last, perf_mode=PM_DR)

        def attn_group(j, qc, ktp, psU_A, psU_B):
            pT2 = attn_qk_exp(j, qc, ktp)
            attn_u(j, ktp, pT2, psU_A, psU_B)

        def attn_finish_copies(psU_A, psU_B):
            """Copy U out of PSUM fast — frees both accumulators for the
            next block.  Returns the SBUF copies."""
            ucs = []
            for psU in (psU_A, psU_B):
                uc = work.tile([DH + 1, QC], F32, tag="ucopy", bufs=6,
                               name="uc")
                nc.vector.tensor_copy(uc, psU)
                ucs.append(uc)
            return ucs

        brc_sink = {}

        def normalize_thunks(j, qc, ucs):
            """Per-head softmax normalize emitted later (as fillers inside
            the next block) so its latency hides under ACT-bound stretches."""
            q0 = qc * QC

            def one(uc, h0):
                def t():
                    # custom-DVE ops require base partition 0: copy the
                    # denominator row down before the fast reciprocal
                    d0 = work.tile([1, QC], F32, tag="d0", bufs=4, name="d0")
                    nc.vector.tensor_copy(d0, uc[DH:DH + 1, :])
                    recip = work.tile([1, QC], F32, tag="recip", bufs=6,
                                      name="recip")
                    nc.vector.reciprocal_approx_fast(recip, d0)
                    brc = work.tile([DH, QC], F32, tag="brc", bufs=6,
                                    name="brc")
                    nc.gpsimd.partition_broadcast(brc, recip)
                    nc.vector.tensor_mul(
                        OT[h0:h0 + DH, j, q0:q0 + QC], uc[0:DH, :], brc)
                    brc_sink[(j, qc)] = brc
                return t
            return [one(ucs[0], 0), one(ucs[1], DH)]

        def attn_span(j, qc, ktps, psU, fillers=(), precomputed=()):
            """Emit the ktile-pair groups of one attention block, sprinkling
            `fillers` (deferred work thunks) between groups so the in-order
            PE/DVE do them inside exp-bound stretches.  Returns this block's
            normalize thunks (to be run as fillers of the NEXT block)."""
            fillers = list(fillers)
            for ktp, pT2 in precomputed:
                attn_u(j, ktp, pT2, psU[0], psU[1])
            spacing = max(1, len(ktps) // (len(fillers) + 1))
            gi = 0
            pend = []     # U-DR lags four ktpairs: the in-order PE never
            for ktp in ktps:  # stalls at a U whose exps are still running
                pT2 = attn_qk_exp(j, qc, ktp)
                pend.append((ktp, pT2))
                if len(pend) > 4:
                    kp, pt = pend.pop(0)
                    attn_u(j, kp, pt, psU[0], psU[1])
                gi += 1
                if fillers and gi % spacing == 0:
                    fillers.pop(0)()
            for kp, pt in pend:
                attn_u(j, kp, pt, psU[0], psU[1])
            for f in fillers:
                f()
            if ktps[-1] == SK_T // 2 - 1:
                ucs = attn_finish_copies(psU[0], psU[1])
                return normalize_thunks(j, qc, ucs), ucs
            return [], None

        def new_psU():
            a = psum.tile([DH + 1, QC], F32, tag="psU", name="psU_A")
            b = psum.tile([DH + 1, QC], F32, tag="psU", name="psU_B")
            return (a, b)

        def outproj(qt):
            # bias matmul first: it reads OT1, whose re-write after the last
            # normalize acts as a scheduling gate for the whole chain (the
            # scheduler otherwise hoists these into mid-attention PE-idle
            # slots and stalls on under-modeled reciprocal latency)
            psO = psum.tile([P, D], F32, tag="ps3", bufs=3, name="psO")
            nc.tensor.matmul(psO, OT1[:, qt * P:(qt + 1) * P],
                             b_row["bo"], start=True, stop=False)
            for j in range(DT):
                nc.tensor.matmul(psO, OT[:, j, qt * P:(qt + 1) * P],
                                 w_bf["wo"][:, j, :],
                                 start=False, stop=(j == DT - 1))
            o_sb = work.tile([P, D], F32, tag="osb", bufs=4, name="o_sb")
            nc.scalar.activation(o_sb, psO, AF.Relu)
            nc.sync.dma_start(out[qt * P:(qt + 1) * P, :], o_sb)

        def gate_outproj(blk):
            """No-op rewrite of OT1 (max(1, recip<1) == 1) that depends on
            block `blk`'s normalize chain — gates the outproj chains (which
            start with an OT1-reading bias matmul) behind it, preventing the
            scheduler from hoisting them into mid-attention stalls."""
            brc = brc_sink[blk]
            nc.vector.tensor_scalar(OT1, OT1, brc[0:1, 0:1], None,
                                    op0=ALU.max)

        # ---- chunk loop: x load + V proj + K proj(pair 0) + attn(0, 0) ----
        psU0 = new_psU()
        N_STORE = 12
        store01 = []
        pendq = []   # queue of deferred normalize-thunk lists (2-block lag)
        for n in range(NCH):
            if n > 0:
                nc.sync.dma_start(xT[:, n], xT_src[:, n])
            kproj(0, n)
            kts = list(range(n * CH, (n + 1) * CH))
            ktps = list(range(n * CH // 2, (n + 1) * CH // 2))
            # QK + exp first: ACT can start before V exists (only U needs V)
            pTs = [(ktp, attn_qk_exp(0, 0, ktp)) for ktp in ktps]
            for st in kts:
                vproj(st)
            for ktp, pT2 in pTs:
                attn_u(0, ktp, pT2, psU0[0], psU0[1])
            if NQC > 1 and n < N_STORE:
                # pre-compute one ktile-pair of block (0,1) per chunk into
                # held pTs: fills the otherwise-idle ACT during the PE-bound
                # chunk phase (the U matmuls run later, so no PSUM cost)
                store01.append((n, attn_qk_exp(0, 1, n, pt_tag="pT01",
                                               pt_bufs=N_STORE, eng="act")))
            if kts[-1] == SK_T - 1:
                ucs0 = attn_finish_copies(psU0[0], psU0[1])
                thunks = normalize_thunks(0, 0, ucs0)
        pendq.append(thunks)

        # ---- remaining attention; fillers inside each ACT-bound block are:
        # the previous block's normalize chain + the next block's
        # projections (+ the qc0 half of the output projection during the
        # last block) ----
        blocks = [(0, qc) for qc in range(1, NQC)]
        blocks += [(j, qc) for j in range(1, DT) for qc in range(NQC)]
        owed = {blk: [] for blk in blocks}
        for (j, qc) in blocks:
            if (j, qc) != (0, 1):
                owed[(j, qc)].append(lambda j=j, qc=qc: qproj(j, qc))
            if qc == 0 and j >= 1:
                for n in range(NCH):
                    owed[(j, qc)].append(lambda j=j, n=n: kproj(j, n))
        for f in owed[blocks[0]]:
            f()
        for bi, (j, qc) in enumerate(blocks):
            # projection fillers first; normalize chains run with a 2-block
            # lag so their slow DVE reciprocals never sit near a block
            # boundary (where they would delay the relus feeding the next
            # pair's attention)
            fillers = []
            if bi + 1 < len(blocks):
                fillers += owed[blocks[bi + 1]]
            last = bi == len(blocks) - 1
            if last:
                # flush remaining normalize chains, then gate + emit the qc0
                # half of the output projection so it runs inside this block
                while pendq:
                    fillers += pendq.pop(0)
                if NQC > 1:
                    fillers += [lambda: gate_outproj((DT - 1, 0))]
                    fillers += [lambda qt=qt: outproj(qt)
                                for qt in range(SQ_T // NQC)]
            elif len(pendq) >= 1:
                fillers += pendq.pop(0)
                if bi == len(blocks) - 2 and pendq:
                    fillers += pendq.pop(0)
            psU = new_psU()
            if (j, qc) == (0, 1) and store01:
                thunks, ucs = attn_span(
                    j, qc, list(range(len(store01), SK_T // 2)), psU,
                    fillers, precomputed=store01)
            else:
                thunks, ucs = attn_span(j, qc, list(range(SK_T // 2)), psU,
                                        fillers)
            pendq.append(thunks)
            last_ucs = ucs

        # ---- tail: last block's normalize + remaining output rows ----
        # Two of the final outproj chains are gated only on the last block's
        # PSUM copies (their bias + pairs-0..2 matmuls need nothing newer),
        # so the PE does useful work during the slow reciprocal chain and
        # stays HAM-warm; their pair-3 matmul still waits on the real OT
        # write.  Gate writes go on DVE BEFORE the normalize thunks so they
        # are not queued behind the reciprocals.
        qt_lo = SQ_T // NQC if NQC > 1 else 0
        early = []
        open_psO = []
        if NQC > 1 and last_ucs is not None:
            early = [qt_lo, qt_lo + 1, qt_lo + 2]
            for qt, uc in zip(early, list(last_ucs) * 2):
                nc.vector.tensor_scalar(
                    OT1[:, qt * P:(qt + 1) * P],
                    OT1[:, qt * P:(qt + 1) * P],
                    uc[DH:DH + 1, 0:1], None, op0=ALU.min)
            # partial chains (bias + pairs 0..2): no pair-3 matmul yet, so
            # the in-order PE runs all 8 matmuls during the reciprocals
            # instead of stalling at the first chain's pair-3 wait
            for qt in early:
                psO = psum.tile([P, D], F32, tag="ps3", bufs=3, name="psO")
                nc.tensor.matmul(psO, OT1[:, qt * P:(qt + 1) * P],
                                 b_row["bo"], start=True, stop=False)
                for j in range(DT - 1):
                    nc.tensor.matmul(psO, OT[:, j, qt * P:(qt + 1) * P],
                                     w_bf["wo"][:, j, :],
                                     start=False, stop=False)
                open_psO.append((qt, psO))
        while pendq:
            for f in pendq.pop(0):
                f()
        for qt, psO in open_psO:
            nc.tensor.matmul(psO, OT[:, DT - 1, qt * P:(qt + 1) * P],
                             w_bf["wo"][:, DT - 1, :],
                             start=False, stop=True)
            o_sb = work.tile([P, D], F32, tag="osb", bufs=4, name="o_sb")
            nc.scalar.activation(o_sb, psO, AF.Relu)
            nc.sync.dma_start(out[qt * P:(qt + 1) * P, :], o_sb)
        gate_outproj(blocks[-1])
        for qt in range(qt_lo, SQ_T):
            if qt not in early:
                outproj(qt)


_NC_CACHE = {}


def _get_nc(sk=S, sq=SQ_FULL, skip_vbias=False):
    key = (sk, sq, skip_vbias)
    if key not in _NC_CACHE:
        _NC_CACHE[key] = build_mha(sk, sq, skip_vbias)
    return _NC_CACHE[key]


def _tile_rows(a):
    """[D, n] -> SBUF layout [P, DT*n]: partition p gets rows p, 128+p, ..."""
    Dd, n = a.shape
    t = Dd // P
    return np.ascontiguousarray(
        a.reshape(t, P, n).transpose(1, 0, 2).reshape(P, t * n))


def _tile_chunks(a, chp):
    """[D, sk] -> chunk-major SBUF layout [P, NCH*DT*chp]: per partition,
    sequence chunks outermost so each chunk is one contiguous linear DMA."""
    Dd, sk = a.shape
    t, nch = Dd // P, sk // chp
    return np.ascontiguousarray(
        a.reshape(t, P, nch, chp).transpose(1, 2, 0, 3).reshape(P, -1))


def prep_inputs(x, Wq, bq, Wk, bk, Wv, bv, Wo, bo):
    """Host-side sharding/layout prep: fp8/bf16 casts, feature-major
    transpose, SBUF pre-tiling.  Returns the 8 per-core input maps."""
    bf = ml_dtypes.bfloat16
    f8 = ml_dtypes.float8_e4m3
    x = np.asarray(x, dtype=np.float32)
    shared = {
        "wq": _tile_rows(np.asarray(Wq, np.float32).astype(f8)),
        "wk": _tile_rows(np.asarray(Wk, np.float32).astype(f8)),
        "wv": _tile_rows(np.asarray(Wv, np.float32).astype(f8)),
        "wo": _tile_rows(np.asarray(Wo, np.float32).astype(bf)),
        "bq": np.ascontiguousarray(
            np.asarray(bq, np.float32).reshape(DT, P).T),
        "bk": np.ascontiguousarray(
            np.asarray(bk, np.float32).reshape(DT, P).T),
        "bv": np.asarray(bv, np.float32).astype(bf).reshape(1, D),
        "bo": np.asarray(bo, np.float32).astype(bf).reshape(1, D),
    }
    xT_b = [x[b].T.astype(f8) for b in range(B)]
    xT_tiled = [_tile_chunks(xb, 4 * P) for xb in xT_b]
    in_maps = []
    for c in range(NCORES):
        b, qo = divmod(c, QSPLIT)
        m = dict(shared)
        m["xT_bf"] = xT_tiled[b]
        m["xqT_bf"] = _tile_rows(
            xT_b[b][:, qo * SQ_FULL:(qo + 1) * SQ_FULL])
        in_maps.append(m)
    return in_maps


def kernel(x, Wq, bq, Wk, bk, Wv, bv, Wo, bo, **run_kwargs):
    """Full-input entry point: shards across 8 NeuronCores, returns full out."""
    in_maps = prep_inputs(x, Wq, bq, Wk, bk, Wv, bv, Wo, bo)
    nc = _get_nc(skip_vbias=bool(np.all(np.asarray(bv) == 0)))
    res = bass_utils.run_bass_kernel_spmd(
        nc, in_maps, core_ids=list(range(NCORES)), **run_kwargs)
    full = np.empty((B, S, D), np.float32)
    for c in range(NCORES):
        b, qo = divmod(c, QSPLIT)
        full[b, qo * SQ_FULL:(qo + 1) * SQ_FULL] = res.results[c]["out"]
    if run_kwargs:
        return full, res
    return full



# revision 60
# speedup vs baseline: 1.0082x; 1.0002x over previous
"""Trainium2 Bass kernel for nn_MultiHeadAttention (B=2, S=4096, D=512, H=8).

Computes: q/k/v = relu(x@W+b) per head, softmax(q k^T / sqrt(64)) v,
out = relu(concat_heads @ Wo + bo).

Sharding: 8 cores = 2 (batch) x 4 (query-slice).  Each core computes full
K/V projections for its batch (redundant across the 4 q-slice cores) and
attention + output projection for its 1024-row query slice.  No collectives;
the host concatenates the 8 output slices.

Host-side prep (part of the sharding/layout step, not device compute):
x and Wq/Wk/Wv are cast to fp8 e4m3 (x also transposed feature-major),
Wo to bf16, all pre-tiled to exact SBUF layout.

Per-core kernel:
  - Q/K/V projections are fp8 DoubleRow matmuls (2 fp8 weights/cell,
    contraction 256 = 128 partitions x 2 kt): half the matmul+LDW count
    of bf16.  Bias+relu fused on DVE -> bf16 K^T/Q^T, fp8 V.
  - scores^T = K^T_h.T @ Q^T_h per (head, ktile): bf16, K=64 contraction;
    the two heads of a pair run concurrently in different PE row-groups
    (emission interleaves A/B per ktile).
  - exp: the two ktiles of a pair go to DIFFERENT engines so they run
    concurrently: even ktile on ACT (table exp, scale=1/8, bias=-2ln2,
    fp8 out), odd ktile on DVE via a Schraudolph bit-trick exp -- one
    tensor_scalar: e4m3(exp(s/8)/4) ~= bitcast_e4m3(int8(A8*s + B8))
    (DVE rounds fp32->int8 to nearest; ~7.5%/elem max err that cancels
    through the softmax ratio to ~1e-3 end-to-end; the /4 keeps
    exp(6.6) in e4m3 range and cancels in the ratio too).  Every 3rd
    odd ktile goes to ACT instead to balance engine load.  Both write
    halves of a shared per-ktpair fp8 tile pT2 [p, ko=2, (A|B)].
  - U^T[65, q] = DoubleRow fp8 matmul per (head, ktpair): lhsT =
    V8[:, 2ktp:2ktp+2, h, 0:65] (ones column 64 = softmax denominator
    row; V8 padded to 66 so the ko step is 16-aligned per the LDW ISA
    rule), rhs = pT2[:, :, h-half]; accumulated over 16 ktpairs in PSUM.
    U matmuls lag one ktpair behind their exps so the in-order PE never
    stalls at a U whose exps are still running.
  - PSUM: scores/projection tiles share one 3-slot rotation ("ps3",
    3 x 2 banks) + 2 psU accumulator banks = all 8 banks; 3 slots keep
    the scores->exp pipeline deep enough to hide exp latency.
  - block end: U copied to SBUF (frees psU), then normalize runs with a
    1-block lag: denominator row copied to partition 0 (custom-DVE ops
    need base partition 0), reciprocal_approx_fast (single DVE op,
    ~5x faster than the iterative reciprocal), gpsimd partition
    broadcast, DVE multiply into feature-major O^T.
  - out = relu(O^T.T @ Wo + bo) in bf16 (fp8 here would cost ~1.8e-2
    rel err -- no softmax cancellation after the output projection);
    bias via ones-row matmul; relu on ACT; DMA to HBM.  OT1-gating and
    partial early chains keep the PE busy through the tail.
"""

import numpy as np
import ml_dtypes

import concourse.bass as bass
import concourse.mybir as mybir
import concourse.tile as tile
from concourse import bacc
from concourse import bass_utils

F32 = mybir.dt.float32
BF16 = mybir.dt.bfloat16
I16 = mybir.dt.int16
I8 = mybir.dt.int8
F8 = mybir.dt.float8e4
AF = mybir.ActivationFunctionType
ALU = mybir.AluOpType
PM_DR = mybir.MatmulPerfMode.DoubleRow

LN2 = float(np.log(2.0))
# Schraudolph-style exp on DVE, direct to fp8 e4m3 (IEEE, bias 7, max 240):
# e4m3(exp(s/8)/4) ~= bitcast_e4m3(int8(A8*s + B8)).  The DVE tensor_scalar
# rounds fp32->int8 to nearest (HW-verified); max rel err ~7.5%/elem, which
# cancels to ~1e-3 end-to-end through the softmax ratio (numerator and
# denominator share the fp8 pT).  The /4 scale keeps exp(6.6) inside e4m3
# range and cancels in the softmax ratio too.
EXP_A8 = 1.0 / LN2            # 8 * (1/8) / ln2
EXP_B8 = 56.0 - 16.0 - 0.344  # 7*8 bias, -16 for the /4 scale, centering
P = 128
D = 512
H = 8
DH = 64
DT = D // P  # 4 (also = number of head pairs)
B = 2
S = 4096
NCORES = 8
QSPLIT = 4
SQ_FULL = S // QSPLIT  # 1024 query rows per core
QC = 512               # q-chunk (matmul free dim / PSUM bank width)


def build_mha(sk=S, sq=SQ_FULL, skip_vbias=False):
    """Build the SPMD Bass program (identical on all cores).

    All inputs arrive pre-tiled by the host into exact SBUF layout
    ([128 partitions, contiguous free bytes]) so every load is a max-packet
    linear DMA."""
    nc = bacc.Bacc("TRN2", target_bir_lowering=False, debug=False,
                   num_devices=NCORES)

    xT_d = nc.dram_tensor("xT_bf", (P, DT * sk), F8,
                          kind="ExternalInput").ap()  # chunk-major, see prep
    xqT_d = nc.dram_tensor("xqT_bf", (P, DT * sq), F8,
                           kind="ExternalInput").ap()
    w_dram = {}
    for n in ("wq", "wk", "wv"):
        w_dram[n] = nc.dram_tensor(n, (P, DT * D), F8,
                                   kind="ExternalInput").ap()
    w_dram["wo"] = nc.dram_tensor("wo", (P, DT * D), BF16,
                                  kind="ExternalInput").ap()
    b_dram = {
        "bq": nc.dram_tensor("bq", (P, DT), F32, kind="ExternalInput").ap(),
        "bk": nc.dram_tensor("bk", (P, DT), F32, kind="ExternalInput").ap(),
        "bv": nc.dram_tensor("bv", (1, D), BF16, kind="ExternalInput").ap(),
        "bo": nc.dram_tensor("bo", (1, D), BF16, kind="ExternalInput").ap(),
    }
    out = nc.dram_tensor("out", (sq, D), F32, kind="ExternalOutput").ap()

    with tile.TileContext(nc) as tc:
        _build_tile(tc, xT_d, xqT_d, w_dram, b_dram, out, sk, sq,
                    skip_vbias)

    nc.compile()
    return nc


def _build_tile(tc, xT_d, xqT_d, w_dram, b_dram, out, sk, sq,
                skip_vbias=False):
    nc = tc.nc
    SK_T = sk // P            # ktiles of the key/value sequence
    SQ_T = sq // P
    NQC = sq // QC            # q chunks per core
    CH = min(4, SK_T)         # stiles per projection chunk
    NCH = SK_T // CH
    KG = 1                    # ktiles per exp group

    with (
        tc.tile_pool(name="singles", bufs=1) as singles,
        tc.tile_pool(name="work", bufs=3) as work,
        tc.tile_pool(name="psum", bufs=2, space="PSUM") as psum,
    ):
        # ---- startup: only what Q-proj pair 0 needs, first ----
        w_bf = {}
        w_bf["wq"] = singles.tile([P, DT, D], F8, name="wq_bf")
        nc.sync.dma_start(w_bf["wq"], w_dram["wq"].rearrange(
            "p (t n) -> p t n", t=DT))
        b_col = {}
        b_col["bq"] = singles.tile([P, DT], F32, name="bq_col")
        nc.sync.dma_start(b_col["bq"], b_dram["bq"])
        xTq = singles.tile([P, DT, sq], F8)
        nc.sync.dma_start(xTq, xqT_d.rearrange("p (t s) -> p t s", t=DT))

        QT = singles.tile([P, DT, sq], BF16)

        def qproj(j, nq):
            psQ = psum.tile([P, QC], F32, tag="ps3", bufs=3, name="psQ")
            for t in range(DT // 2):
                nc.tensor.matmul(
                    psQ, w_bf["wq"][:, 2 * t:2 * t + 2, j * P:(j + 1) * P],
                    xTq[:, 2 * t:2 * t + 2, nq * QC:(nq + 1) * QC],
                    start=(t == 0), stop=(t == DT // 2 - 1),
                    perf_mode=PM_DR)
            nc.vector.tensor_scalar(
                QT[:, j, nq * QC:(nq + 1) * QC], psQ,
                b_col["bq"][:, j:j + 1], 0.0, op0=ALU.add, op1=ALU.max)

        qproj(0, 0)
        if NQC > 1:
            qproj(0, 1)

        # ---- K-proj deps next (attention can start before V exists) ----
        b_row = {}
        w_bf["wk"] = singles.tile([P, DT, D], F8, name="wk_bf")
        nc.sync.dma_start(w_bf["wk"], w_dram["wk"].rearrange(
            "p (t n) -> p t n", t=DT))
        b_col["bk"] = singles.tile([P, DT], F32, name="bk_col")
        nc.sync.dma_start(b_col["bk"], b_dram["bk"])
        CHP = CH * P
        xT = singles.tile([P, NCH, DT, CHP], F8)
        xT_src = xT_d.rearrange("p (n t s) -> p n t s", n=NCH, t=DT)
        nc.sync.dma_start(xT[:, 0], xT_src[:, 0])
        for n in ("wv", "wo"):
            wb = singles.tile([P, DT, D], F8 if n == "wv" else BF16,
                              name=f"{n}_bf")
            nc.sync.dma_start(wb, w_dram[n].rearrange(
                "p (t n) -> p t n", t=DT))
            w_bf[n] = wb
            if n == "wv":
                br = singles.tile([1, D], BF16, name="bv_row")
                nc.sync.dma_start(br, b_dram["bv"])
                b_row["bv"] = br
        br = singles.tile([1, D], BF16, name="bo_row")
        nc.sync.dma_start(br, b_dram["bo"])
        b_row["bo"] = br

        # ---- persistent SBUF tensors ----
        xT1 = singles.tile([1, sk], BF16)
        nc.vector.memset(xT1, 1.0)
        KT = singles.tile([P, DT, sk], BF16)
        # V in fp8 e4m3 for the DoubleRow U matmul.  Layout [p, st, h, 66]:
        # col 64 = ones (denominator row), col 65 = pad so the DoubleRow
        # ktile-pair step (8*66 = 528 elems) is 16-aligned per the LDW ISA
        # restriction.
        MV = DH + 2
        V8 = singles.tile([P, SK_T, H, MV], F8, name="V8")
        nc.vector.memset(V8[:, :, :, DH:DH + 1], 1.0)
        nl2m = singles.tile([P, 1], F32, name="nl2m")
        nc.vector.memset(nl2m, -2.0 * LN2)  # ACT exp bias: the /4 scale
        OT = singles.tile([P, DT, sq], BF16)
        OT1 = singles.tile([1, sq], BF16)
        nc.vector.memset(OT1, 1.0)

        # PSUM tags: "proj" 2x1 banks, "scores" 2x2 banks, "psU" 2x1 = 8
        def vproj(st):
            n, si = st // CH, st % CH
            psV = psum.tile([P, D], F32, tag="ps3", bufs=3, name="psV")
            for t in range(DT // 2):
                nc.tensor.matmul(
                    psV, xT[:, n, 2 * t:2 * t + 2, si * P:(si + 1) * P],
                    w_bf["wv"][:, 2 * t:2 * t + 2, :],
                    start=(t == 0),
                    stop=(skip_vbias and t == DT // 2 - 1),
                    perf_mode=PM_DR)
            if not skip_vbias:
                nc.tensor.matmul(psV, xT1[:, st * P:(st + 1) * P],
                                 b_row["bv"], start=False, stop=True)
            nc.vector.tensor_scalar_max(
                V8[:, st, :, 0:DH],
                psV.rearrange("p (h d) -> p h d", h=H), 0.0)

        def kproj(j, n):
            psK = psum.tile([P, CH * P], F32, tag="ps3", bufs=3, name="psK")
            for t in range(DT // 2):
                nc.tensor.matmul(
                    psK, w_bf["wk"][:, 2 * t:2 * t + 2, j * P:(j + 1) * P],
                    xT[:, n, 2 * t:2 * t + 2, :],
                    start=(t == 0), stop=(t == DT // 2 - 1),
                    perf_mode=PM_DR)
            nc.vector.tensor_scalar(
                KT[:, j, n * CH * P:(n + 1) * CH * P], psK,
                b_col["bk"][:, j:j + 1], 0.0, op0=ALU.add, op1=ALU.max)

        exp_state = {"g": 0}

        def attn_qk_exp(j, qc, ktp, pt_tag="pT", pt_bufs=9, eng=None):
            """Scores + exp for ktile pair (2*ktp, 2*ktp+1) x 2 heads.
            Emission matches the pre-DoubleRow kernel: per ktile, one psS
            [P, A|B] with the two heads' matmuls adjacent (PE row-group
            pairing), one exp op [P, 1024].  The exp output lands in a
            shared per-ktpair fp8 tile pT2 [P, ko=2, (A|B)] so each head's
            DoubleRow rhs is the strided view pT2[:, :, h-half]."""
            q0 = qc * QC
            # byte-interleaved pair layout [p, q, ko]: the DR rhs stream
            # fetches the 2 fp8 values of a cell as one 16-bit read
            pT2 = work.tile([P, 2 * QC, 2], F8, tag=pt_tag,
                            bufs=pt_bufs, name="pT2")
            for i in (0, 1):
                kt = 2 * ktp + i
                psS = psum.tile([P, 2 * QC], F32, tag="ps3", bufs=3,
                                name="psS")
                nc.tensor.matmul(
                    psS[:, 0:QC],
                    KT[0:DH, j, kt * P:(kt + 1) * P],
                    QT[0:DH, j, q0:q0 + QC], start=True, stop=True)
                nc.tensor.matmul(
                    psS[:, QC:2 * QC],
                    KT[DH:P, j, kt * P:(kt + 1) * P],
                    QT[DH:P, j, q0:q0 + QC], start=True, stop=True)
                if eng is not None:
                    e = eng
                elif i == 0:
                    e = "act"
                else:
                    g = exp_state["g"]
                    exp_state["g"] = g + 1
                    e = "act" if (g % 3) == 2 else "dve"
                if e == "dve":
                    nc.vector.tensor_scalar(
                        pT2[:, :, i].bitcast(I8), psS,
                        EXP_A8, EXP_B8, op0=ALU.mult, op1=ALU.add)
                else:
                    nc.scalar.activation(pT2[:, :, i], psS, AF.Exp,
                                         scale=0.125, bias=nl2m)
            return pT2

        def attn_u(j, ktp, pT2, psU_A, psU_B):
            """DoubleRow fp8 matmul: contraction over both ktiles of the
            pair (128 partitions x ko=2)."""
            first, last = (ktp == 0), (ktp == SK_T // 2 - 1)
            for h2, psU in ((0, psU_A), (1, psU_B)):
                nc.tensor.matmul(
                    psU, V8[:, 2 * ktp:2 * ktp + 2, 2 * j + h2, 0:DH + 1],
                    pT2[:, h2 * QC:(h2 + 1) * QC, :].transpose([0, 2, 1]),
                    start=first, stop=last, perf_mode=PM_DR)

        def attn_group(j, qc, ktp, psU_A, psU_B):
            pT2 = attn_qk_exp(j, qc, ktp)
            attn_u(j, ktp, pT2, psU_A, psU_B)

        def attn_finish_copies(psU_A, psU_B):
            """Copy U out of PSUM fast — frees both accumulators for the
            next block.  Returns the SBUF copies."""
            ucs = []
            for psU in (psU_A, psU_B):
                uc = work.tile([DH + 1, QC], F32, tag="ucopy", bufs=6,
                               name="uc")
                nc.vector.tensor_copy(uc, psU)
                ucs.append(uc)
            return ucs

        brc_sink = {}

        def normalize_thunks(j, qc, ucs):
            """Per-head softmax normalize emitted later (as fillers inside
            the next block) so its latency hides under ACT-bound stretches."""
            q0 = qc * QC

            def one(uc, h0):
                def t():
                    # custom-DVE ops require base partition 0: copy the
                    # denominator row down before the fast reciprocal
                    d0 = work.tile([1, QC], F32, tag="d0", bufs=2, name="d0")
                    nc.vector.tensor_copy(d0, uc[DH:DH + 1, :])
                    recip = work.tile([1, QC], F32, tag="recip", bufs=4,
                                      name="recip")
                    nc.vector.reciprocal_approx_fast(recip, d0)
                    brc = work.tile([DH, QC], F32, tag="brc", bufs=4,
                                    name="brc")
                    nc.gpsimd.partition_broadcast(brc, recip)
                    nc.vector.tensor_mul(
                        OT[h0:h0 + DH, j, q0:q0 + QC], uc[0:DH, :], brc)
                    brc_sink[(j, qc)] = brc
                return t
            return [one(ucs[0], 0), one(ucs[1], DH)]

        def attn_span(j, qc, ktps, psU, fillers=(), precomputed=()):
            """Emit the ktile-pair groups of one attention block, sprinkling
            `fillers` (deferred work thunks) between groups so the in-order
            PE/DVE do them inside exp-bound stretches.  Returns this block's
            normalize thunks (to be run as fillers of the NEXT block)."""
            fillers = list(fillers)
            for ktp, pT2 in precomputed:
                attn_u(j, ktp, pT2, psU[0], psU[1])
            spacing = max(1, len(ktps) // (len(fillers) + 1))
            gi = 0
            pend = []     # U-DR lags four ktpairs: the in-order PE never
            for ktp in ktps:  # stalls at a U whose exps are still running
                pT2 = attn_qk_exp(j, qc, ktp)
                pend.append((ktp, pT2))
                if len(pend) > 4:
                    kp, pt = pend.pop(0)
                    attn_u(j, kp, pt, psU[0], psU[1])
                gi += 1
                if fillers and gi % spacing == 0:
                    fillers.pop(0)()
            for kp, pt in pend:
                attn_u(j, kp, pt, psU[0], psU[1])
            for f in fillers:
                f()
            if ktps[-1] == SK_T // 2 - 1:
                ucs = attn_finish_copies(psU[0], psU[1])
                return normalize_thunks(j, qc, ucs), ucs
            return [], None

        def new_psU():
            a = psum.tile([DH + 1, QC], F32, tag="psU", name="psU_A")
            b = psum.tile([DH + 1, QC], F32, tag="psU", name="psU_B")
            return (a, b)

        def outproj(qt):
            # bias matmul first: it reads OT1, whose re-write after the last
            # normalize acts as a scheduling gate for the whole chain (the
            # scheduler otherwise hoists these into mid-attention PE-idle
            # slots and stalls on under-modeled reciprocal latency)
            psO = psum.tile([P, D], F32, tag="ps3", bufs=3, name="psO")
            nc.tensor.matmul(psO, OT1[:, qt * P:(qt + 1) * P],
                             b_row["bo"], start=True, stop=False)
            for j in range(DT):
                nc.tensor.matmul(psO, OT[:, j, qt * P:(qt + 1) * P],
                                 w_bf["wo"][:, j, :],
                                 start=False, stop=(j == DT - 1))
            o_sb = work.tile([P, D], F32, tag="osb", bufs=4, name="o_sb")
            nc.scalar.activation(o_sb, psO, AF.Relu)
            nc.sync.dma_start(out[qt * P:(qt + 1) * P, :], o_sb)

        def gate_outproj(blk):
            """No-op rewrite of OT1 (max(1, recip<1) == 1) that depends on
            block `blk`'s normalize chain — gates the outproj chains (which
            start with an OT1-reading bias matmul) behind it, preventing the
            scheduler from hoisting them into mid-attention stalls."""
            brc = brc_sink[blk]
            nc.vector.tensor_scalar(OT1, OT1, brc[0:1, 0:1], None,
                                    op0=ALU.max)

        # ---- chunk loop: x load + V proj + K proj(pair 0) + attn(0, 0) ----
        psU0 = new_psU()
        N_STORE = 12
        store01 = []
        pendq = []   # queue of deferred normalize-thunk lists (2-block lag)
        for n in range(NCH):
            if n > 0:
                nc.sync.dma_start(xT[:, n], xT_src[:, n])
            kproj(0, n)
            kts = list(range(n * CH, (n + 1) * CH))
            ktps = list(range(n * CH // 2, (n + 1) * CH // 2))
            # QK + exp first: ACT can start before V exists (only U needs V)
            pTs = [(ktp, attn_qk_exp(0, 0, ktp)) for ktp in ktps]
            for st in kts:
                vproj(st)
            for ktp, pT2 in pTs:
                attn_u(0, ktp, pT2, psU0[0], psU0[1])
            if NQC > 1 and n < N_STORE:
                # pre-compute one ktile-pair of block (0,1) per chunk into
                # held pTs: fills the otherwise-idle ACT during the PE-bound
                # chunk phase (the U matmuls run later, so no PSUM cost)
                store01.append((n, attn_qk_exp(0, 1, n, pt_tag="pT01",
                                               pt_bufs=N_STORE, eng="act")))
            if kts[-1] == SK_T - 1:
                ucs0 = attn_finish_copies(psU0[0], psU0[1])
                thunks = normalize_thunks(0, 0, ucs0)
        pendq.append(thunks)

        # ---- remaining attention; fillers inside each ACT-bound block are:
        # the previous block's normalize chain + the next block's
        # projections (+ the qc0 half of the output projection during the
        # last block) ----
        blocks = [(0, qc) for qc in range(1, NQC)]
        blocks += [(j, qc) for j in range(1, DT) for qc in range(NQC)]
        owed = {blk: [] for blk in blocks}
        for (j, qc) in blocks:
            if (j, qc) != (0, 1):
                owed[(j, qc)].append(lambda j=j, qc=qc: qproj(j, qc))
            if qc == 0 and j >= 1:
                for n in range(NCH):
                    owed[(j, qc)].append(lambda j=j, n=n: kproj(j, n))
        for f in owed[blocks[0]]:
            f()
        for bi, (j, qc) in enumerate(blocks):
            # projection fillers first; normalize chains run with a 2-block
            # lag so their slow DVE reciprocals never sit near a block
            # boundary (where they would delay the relus feeding the next
            # pair's attention)
            fillers = []
            if bi + 1 < len(blocks):
                fillers += owed[blocks[bi + 1]]
            last = bi == len(blocks) - 1
            if last:
                # flush remaining normalize chains, then gate + emit the qc0
                # half of the output projection so it runs inside this block
                while pendq:
                    fillers += pendq.pop(0)
                if NQC > 1:
                    fillers += [lambda: gate_outproj((DT - 1, 0))]
                    fillers += [lambda qt=qt: outproj(qt)
                                for qt in range(SQ_T // NQC)]
            elif len(pendq) >= 1:
                fillers += pendq.pop(0)
                if bi == len(blocks) - 2 and pendq:
                    fillers += pendq.pop(0)
            psU = new_psU()
            if (j, qc) == (0, 1) and store01:
                thunks, ucs = attn_span(
                    j, qc, list(range(len(store01), SK_T // 2)), psU,
                    fillers, precomputed=store01)
            else:
                thunks, ucs = attn_span(j, qc, list(range(SK_T // 2)), psU,
                                        fillers)
            pendq.append(thunks)
            last_ucs = ucs

        # ---- tail: last block's normalize + remaining output rows ----
        # Two of the final outproj chains are gated only on the last block's
        # PSUM copies (their bias + pairs-0..2 matmuls need nothing newer),
        # so the PE does useful work during the slow reciprocal chain and
        # stays HAM-warm; their pair-3 matmul still waits on the real OT
        # write.  Gate writes go on DVE BEFORE the normalize thunks so they
        # are not queued behind the reciprocals.
        qt_lo = SQ_T // NQC if NQC > 1 else 0
        early = []
        open_psO = []
        if NQC > 1 and last_ucs is not None:
            early = [qt_lo, qt_lo + 1, qt_lo + 2]
            for qt, uc in zip(early, list(last_ucs) * 2):
                nc.vector.tensor_scalar(
                    OT1[:, qt * P:(qt + 1) * P],
                    OT1[:, qt * P:(qt + 1) * P],
                    uc[DH:DH + 1, 0:1], None, op0=ALU.min)
            # partial chains (bias + pairs 0..2): no pair-3 matmul yet, so
            # the in-order PE runs all 8 matmuls during the reciprocals
            # instead of stalling at the first chain's pair-3 wait
            for qt in early:
                psO = psum.tile([P, D], F32, tag="ps3", bufs=3, name="psO")
                nc.tensor.matmul(psO, OT1[:, qt * P:(qt + 1) * P],
                                 b_row["bo"], start=True, stop=False)
                for j in range(DT - 1):
                    nc.tensor.matmul(psO, OT[:, j, qt * P:(qt + 1) * P],
                                     w_bf["wo"][:, j, :],
                                     start=False, stop=False)
                open_psO.append((qt, psO))
        while pendq:
            for f in pendq.pop(0):
                f()
        for qt, psO in open_psO:
            nc.tensor.matmul(psO, OT[:, DT - 1, qt * P:(qt + 1) * P],
                             w_bf["wo"][:, DT - 1, :],
                             start=False, stop=True)
            o_sb = work.tile([P, D], F32, tag="osb", bufs=4, name="o_sb")
            nc.scalar.activation(o_sb, psO, AF.Relu)
            nc.sync.dma_start(out[qt * P:(qt + 1) * P, :], o_sb)
        gate_outproj(blocks[-1])
        for qt in range(qt_lo, SQ_T):
            if qt not in early:
                outproj(qt)


_NC_CACHE = {}


def _get_nc(sk=S, sq=SQ_FULL, skip_vbias=False):
    key = (sk, sq, skip_vbias)
    if key not in _NC_CACHE:
        _NC_CACHE[key] = build_mha(sk, sq, skip_vbias)
    return _NC_CACHE[key]


def _tile_rows(a):
    """[D, n] -> SBUF layout [P, DT*n]: partition p gets rows p, 128+p, ..."""
    Dd, n = a.shape
    t = Dd // P
    return np.ascontiguousarray(
        a.reshape(t, P, n).transpose(1, 0, 2).reshape(P, t * n))


def _tile_chunks(a, chp):
    """[D, sk] -> chunk-major SBUF layout [P, NCH*DT*chp]: per partition,
    sequence chunks outermost so each chunk is one contiguous linear DMA."""
    Dd, sk = a.shape
    t, nch = Dd // P, sk // chp
    return np.ascontiguousarray(
        a.reshape(t, P, nch, chp).transpose(1, 2, 0, 3).reshape(P, -1))


def prep_inputs(x, Wq, bq, Wk, bk, Wv, bv, Wo, bo):
    """Host-side sharding/layout prep: fp8/bf16 casts, feature-major
    transpose, SBUF pre-tiling.  Returns the 8 per-core input maps."""
    bf = ml_dtypes.bfloat16
    f8 = ml_dtypes.float8_e4m3
    x = np.asarray(x, dtype=np.float32)
    shared = {
        "wq": _tile_rows(np.asarray(Wq, np.float32).astype(f8)),
        "wk": _tile_rows(np.asarray(Wk, np.float32).astype(f8)),
        "wv": _tile_rows(np.asarray(Wv, np.float32).astype(f8)),
        "wo": _tile_rows(np.asarray(Wo, np.float32).astype(bf)),
        "bq": np.ascontiguousarray(
            np.asarray(bq, np.float32).reshape(DT, P).T),
        "bk": np.ascontiguousarray(
            np.asarray(bk, np.float32).reshape(DT, P).T),
        "bv": np.asarray(bv, np.float32).astype(bf).reshape(1, D),
        "bo": np.asarray(bo, np.float32).astype(bf).reshape(1, D),
    }
    xT_b = [x[b].T.astype(f8) for b in range(B)]
    xT_tiled = [_tile_chunks(xb, 4 * P) for xb in xT_b]
    in_maps = []
    for c in range(NCORES):
        b, qo = divmod(c, QSPLIT)
        m = dict(shared)
        m["xT_bf"] = xT_tiled[b]
        m["xqT_bf"] = _tile_rows(
            xT_b[b][:, qo * SQ_FULL:(qo + 1) * SQ_FULL])
        in_maps.append(m)
    return in_maps


def kernel(x, Wq, bq, Wk, bk, Wv, bv, Wo, bo, **run_kwargs):
    """Full-input entry point: shards across 8 NeuronCores, returns full out."""
    in_maps = prep_inputs(x, Wq, bq, Wk, bk, Wv, bv, Wo, bo)
    nc = _get_nc(skip_vbias=bool(np.all(np.asarray(bv) == 0)))
    res = bass_utils.run_bass_kernel_spmd(
        nc, in_maps, core_ids=list(range(NCORES)), **run_kwargs)
    full = np.empty((B, S, D), np.float32)
    for c in range(NCORES):
        b, qo = divmod(c, QSPLIT)
        full[b, qo * SQ_FULL:(qo + 1) * SQ_FULL] = res.results[c]["out"]
    if run_kwargs:
        return full, res
    return full

